# revision 45
# baseline (speedup 1.0000x reference)
"""Expert-parallel MoE (Mixtral-style top-2 of 8 experts, SwiGLU) on 8 TRN2 cores.

Strategy: expert PAIRING with a half-intermediate split. Experts are sorted by
token load and paired big-with-small; the two cores of pair p each process ALL
tokens of both experts, but only half of the intermediate dim I (so weight
bytes per core stay at 1/8 of the total). The host sums the two half-I
down-proj partials per expert. Slot capacities are uniform across cores (SPMD
single program): CA = max big-expert load, CB = max small-expert load — this
equals the packing lower bound l1+l5.

Routing (softmax/top-k/renorm, 1024x8) runs on host during input sharding;
the renormalized routing weight is folded in at host combine time.

Device kernel per core (all matmuls bf16, fp32 PSUM accumulation):
  GU:   for j, for slot in (A,B): g/u[j] = W13T[slot,j] . xt  (16 ko steps)
        act = silu(g)*u  (bf16); 2 PSUM accumulators per slot fold the first
        2 down-proj output chunks in as act[j-1] becomes available
  DOWN: remaining 14 output chunks per slot, B/A interleaved per h-chunk,
        6-wide PSUM passes (reusing the g/u + accumulator banks); output
        written bf16. The interleave keeps every core's w2b stream demand
        steady (~273 GB/s) — an all-B phase peaks at 299 GB/s on all 8
        cores in lockstep, which starved the slowest core's JIT stream,
        stalled its PE >3.4us and tripped the HAM clock-gate back to
        1.2 GHz (exec time is the MAX over cores, so one straggler core
        is the whole metric).

DMA discipline (the per-core limits that shape this kernel):
  - All DMA queues share the same 16 DMA engines (~400 GB/s aggregate);
    concurrently active queues split that roughly evenly, so a sustained
    second stream starves the critical one. Only sync and scalar have HW
    DGE queues; gpsimd's software DGE does ~50 GB/s.
  - Both GU and DOWN consume weights at ~256 GB/s (each weight byte feeds
    one matmul of ~280 columns). Total input is 51 MB against a ~182 us
    matmul stream — the kernel is near BOTH the PE and the single-queue
    DMA roofline, so the sync queue carries ALL weights in exact
    consumption order and must never idle:
      w13 j-tiles (4/j) -> w2b as per-(slot,h) chunk tiles streamed
      just-in-time through DOWN (pool backpressure paces them), slot B
      chunks first to match the DOWN order.
  - The w13 pool holds 16 tiles (4 j-groups of lead) so transient DMA dips
    never stall the PE; w2b-by-h tiles need only 8 bufs (32 KB/partition)
    instead of a fully-resident 112 KB/partition prefetch.
  - The start is slot-A-first so the PE never waits for the full xt: xta
    blocks + A weights (~2.8 MB) feed j0A/j1A while xtb + B weights stream
    behind them; xt rides scalar (active only the first ~10 us), and the
    j0/j1 weight tiles arrive as 8-ko halves so each piece is usable as
    soon as its prefix of the sync stream lands. w2a (per-j 128 KB tiles
    for the folded chunks) and output writebacks ride gpsimd's software
    DGE (~13 GB/s trickle).
  - The first ~10 us are DMA-ramp-bound (all 8 cores slam HBM at once), so
    dep-free warm-up matmuls on a memset tile run upfront (N_WARM) and as
    filler batches inside the j0A gate phase (J0_FILL): in-order PE
    execution places them exactly in the DMA-wait windows, keeping the PE
    busy so the HAM clock-gate holds 2.4 GHz from ~11 us for the whole
    kernel (otherwise ramp gaps re-throttle it to 1.2 GHz).
"""

import os

import ml_dtypes
import numpy as np

import concourse.bass as bass
from concourse import bacc
import concourse.mybir as mybir
import concourse.tile as tile
from concourse.bass_utils import run_bass_kernel_spmd

P = 128
H = 2048          # hidden dim
I = 4096          # intermediate dim
IH = I // 2       # per-core intermediate half
E = 8             # experts
N_CORES = 8
BF16 = mybir.dt.bfloat16
F32 = mybir.dt.float32

KO = H // P       # 16 contraction steps over hidden dim
NJ = IH // P      # 16 j-tiles per slot (half intermediate)
NH = H // P       # 16 output row chunks
NC1 = 2           # down-proj chunks folded into the GU phase per slot
NHD = NH - NC1    # down-proj chunks done in the DOWN phase
XB = 4            # xt ko-blocks (KO/XB per block)
KB = KO // XB     # ko per block (4)
N_WARM = 16       # warm-up dummy matmuls (HAM un-throttle during DMA prime)

# set by kernel() for test harness introspection
last_results = None


def _build_nc(CA: int, CB: int) -> bass.Bass:
    act_fn = mybir.ActivationFunctionType
    CT = CA + CB

    nc = bacc.Bacc()
    xta_d = nc.declare_dram_parameter("xta", [P, KO, CA], BF16, isOutput=False)
    xtb_d = nc.declare_dram_parameter("xtb", [P, KO, CB], BF16, isOutput=False)
    # per slot: [j, kind(0=gate,1=up), P, KO, P]
    w13a_d = nc.declare_dram_parameter("w13a", [NJ, 2, P, KO, P], BF16, isOutput=False)
    w13b_d = nc.declare_dram_parameter("w13b", [NJ, 2, P, KO, P], BF16, isOutput=False)
    # w2a: per-j tiles for the NC1 folded chunks; w2bh: per-(slot, h) tiles
    # holding all NJ contraction slices for one output chunk
    w2a_d = nc.declare_dram_parameter("w2a", [NJ, P, 2, NC1 * P], BF16, isOutput=False)
    w2bh_d = nc.declare_dram_parameter("w2bh", [2, NHD, P, NJ * P], BF16, isOutput=False)
    y_d = nc.declare_dram_parameter("y", [NH, P, CT], BF16, isOutput=True)

    SLOTS = [(0, 0, CA), (1, CA, CB)]  # (slot, col offset, width)
    w13_dram = (w13a_d, w13b_d)

    with tile.TileContext(nc) as tc:
        with (
            tc.tile_pool(name="xtp", bufs=1) as xtp,
            tc.tile_pool(name="w13f", bufs=1) as w13fp,
            tc.tile_pool(name="w13p", bufs=12) as w13p,
            tc.tile_pool(name="w2ap", bufs=4) as w2ap,
            tc.tile_pool(name="w2bp", bufs=16) as w2bp,
            tc.tile_pool(name="actp", bufs=1) as actp,
            tc.tile_pool(name="silup", bufs=2) as silup,
            tc.tile_pool(name="outp", bufs=4) as outp,
            tc.tile_pool(name="dummyp", bufs=1) as dummyp,
            tc.tile_pool(name="psgu", bufs=2, space="PSUM") as psgu,
            tc.tile_pool(name="psacc", bufs=1, space="PSUM") as psacc,
        ):
            # ---- tiles and DMA helpers ------------------------------------
            xta_tiles, xtb_tiles = [], []

            def dma_xta(b):
                sb = xtp.tile([P, KB, CA], BF16, tag=f"xta_{b}", name=f"xta_{b}")
                nc.scalar.dma_start(sb[:], xta_d[:, b * KB:(b + 1) * KB, :])
                assert len(xta_tiles) == b
                xta_tiles.append(sb)

            def dma_xtb(hf):
                kb = KO // 2
                sb = xtp.tile([P, kb, CB], BF16, tag=f"xtb_{hf}", name=f"xtb_{hf}")
                nc.scalar.dma_start(sb[:], xtb_d[:, hf * kb:(hf + 1) * kb, :])
                assert len(xtb_tiles) == hf
                xtb_tiles.append(sb)

            def xt_slice(ko, slot):
                if slot == 0:
                    return xta_tiles[ko // KB][:, ko % KB, :]
                kb = KO // 2
                return xtb_tiles[ko // kb][:, ko % kb, :]

            # j0/j1 tiles are halved (8-ko pieces) so each piece is usable
            # as soon as its prefix of the sync stream lands
            HALVED = {(s, k, j) for s in (0, 1) for k in (0, 1) for j in (0, 1)}
            w13h = {}

            def dma_w13h(slot, kind, j, hf):
                kb = KO // 2
                sb = w13fp.tile([P, kb, P], BF16,
                                tag=f"w13h_{slot}_{kind}_{j}_{hf}",
                                name=f"w13h_{slot}_{kind}_{j}_{hf}")
                nc.sync.dma_start(
                    sb[:], w13_dram[slot][j, kind][:, hf * kb:(hf + 1) * kb, :])
                w13h[(slot, kind, j, hf)] = sb

            w13_tiles = {}

            def dma_w13(slot, kind, j):
                sb = w13p.tile([P, KO, P], BF16, tag="w13",
                               name=f"w13_{slot}_{kind}_{j}")
                nc.sync.dma_start(sb[:], w13_dram[slot][j, kind])
                w13_tiles[(slot, kind, j)] = sb

            w2a_tiles = {}

            def dma_w2a(j):
                sb = w2ap.tile([P, 2, NC1 * P], BF16, tag="w2a", name=f"w2a_{j}")
                nc.gpsimd.dma_start(sb[:], w2a_d[j])
                w2a_tiles[j] = sb

            w2bh_tiles = {}

            def dma_w2bh(slot, h):
                # one tile per DOWN pass; bufs=8 backpressure paces the sync
                # queue just-in-time through the DOWN phase
                sb = w2bp.tile([P, NJ * P], BF16, tag="w2bh", name=f"w2bh_{slot}_{h}")
                nc.sync.dma_start(sb[:], w2bh_d[slot, h - NC1])
                w2bh_tiles[(slot, h)] = sb

            # ---- warm-up tile (memset, no DMA needed) ---------------------
            dummy = dummyp.tile([P, CA], BF16, tag="dummy", name="dummy")
            nc.gpsimd.memset(dummy[:], 0.0)

            # ---- priming -------------------------------------------------
            # scalar: xta blocks then xtb halves (idle afterwards); sync:
            # A-slot j0/j1 weights first, then B-slot, then the j2+ stream
            # in consumption order; gpsimd: w2a per-j tiles.
            dma_xta(0)
            dma_w13h(0, 0, 0, 0)     # Ag j0 first half (256 KB)
            dma_xta(1)
            dma_w13h(0, 0, 0, 1)
            dma_xta(2)
            dma_xta(3)
            dma_w13h(0, 1, 0, 0)     # Au j0 halves
            dma_w13h(0, 1, 0, 1)
            dma_xtb(0)
            dma_w13h(0, 0, 1, 0)     # j1 A halves
            dma_w13h(0, 0, 1, 1)
            dma_w13h(0, 1, 1, 0)
            dma_w13h(0, 1, 1, 1)
            dma_xtb(1)
            for j in (2, 3, 4):      # deep prime; loop prefetches from j5
                dma_w13(0, 0, j)
                dma_w13(0, 1, j)
                dma_w13(1, 0, j)
                dma_w13(1, 1, j)
            # B j0/j1 halves trail the prime: their passes are DEFERRED to
            # mid-GU (after j4), so these land in the steady stream instead
            # of widening the startup DMA crunch
            for (k, j) in ((0, 0), (1, 0), (0, 1), (1, 1)):
                dma_w13h(1, k, j, 0)
                dma_w13h(1, k, j, 1)
            dma_w2a(0)               # gpsimd, per-j 128 KB tiles
            dma_w2a(1)
            dma_w2a(2)

            # persistent PSUM accumulators for the first NC1 output chunks
            acc = {}
            for slot, _, cw in SLOTS:
                for c in range(NC1):
                    acc[(slot, c)] = psacc.tile(
                        [P, cw], F32, tag=f"acc{slot}{c}", name=f"acc_{slot}_{c}"
                    )

            # ---- warm-up dummies: keep PE busy from ~7us so HAM is at
            # 2.4 GHz when the real stream starts; they write acc00 which the
            # j0 fold later resets with start=True.
            for w in range(N_WARM):
                nc.tensor.matmul(
                    acc[(0, 0)][:], dummy[:, 0:P], dummy[:],
                    start=True, stop=True,
                )

            act_tiles = {}

            def silu(slot, j, g_ps, u_ps, cw):
                s_sb = silup.tile([P, cw], F32, tag="s", name=f"s_{slot}_{j}")
                nc.scalar.activation(s_sb[:], g_ps[:], act_fn.Sigmoid)
                su_sb = silup.tile([P, cw], F32, tag="su", name=f"su_{slot}_{j}")
                nc.vector.tensor_mul(su_sb[:], s_sb[:], u_ps[:])
                a_sb = actp.tile([P, cw], BF16, tag=f"act_{slot}_{j}",
                                 name=f"act_{slot}_{j}")
                nc.vector.tensor_mul(a_sb[:], su_sb[:], g_ps[:])
                act_tiles[(slot, j)] = a_sb

            def warm_batch(n):
                # dep-free filler matmuls: in-order PE execution runs them
                # exactly during early DMA waits, keeping HAM at 2.4 GHz
                for _ in range(n):
                    nc.tensor.matmul(
                        acc[(0, 0)][:], dummy[:, 0:P], dummy[:],
                        start=True, stop=True,
                    )

            # filler sizes inside j0A after (kind, ko) — the xt-block and
            # weight-half wait points of the DMA ramp
            J0_FILL = {(0, 3): 14, (0, 7): 8, (0, 11): 5, (0, 15): 5,
                       (1, 3): 6, (1, 11): 4}

            def gu_step(slot, j):
                _, c0, cw = SLOTS[slot]
                g_ps = psgu.tile([P, cw], F32, tag="g", name=f"g_{slot}_{j}")
                u_ps = psgu.tile([P, cw], F32, tag="u", name=f"u_{slot}_{j}")
                kb = KO // 2
                for kind, ps in ((0, g_ps), (1, u_ps)):
                    for ko in range(KO):
                        if (slot, kind, j) in HALVED:
                            w_sl = w13h[(slot, kind, j, ko // kb)][:, ko % kb, :]
                        else:
                            w_sl = w13_tiles[(slot, kind, j)][:, ko, :]
                        nc.tensor.matmul(
                            ps[:], w_sl, xt_slice(ko, slot),
                            start=(ko == 0), stop=(ko == KO - 1),
                        )
                        if slot == 0 and j == 0:
                            warm_batch(J0_FILL.get((kind, ko), 0))
                silu(slot, j, g_ps, u_ps, cw)

            def fold_slot(slot, j, stop=False):
                for c in range(NC1):
                    nc.tensor.matmul(
                        acc[(slot, c)][:],
                        w2a_tiles[j][:, slot, c * P:(c + 1) * P],
                        act_tiles[(slot, j)][:],
                        start=(j == 0),
                        stop=stop,
                    )

            # ---- GU start: slot A only (just xta + A weight halves on the
            # critical path); slot B's j0/j1 are deferred to after j4, by
            # which time the DMA ramp is over and the stream is PE-bound
            gu_step(0, 0)
            gu_step(0, 1)
            fold_slot(0, 0)

            # ---- GU j = 2..NJ-1 ------------------------------------------
            for j in range(2, NJ):
                # prefetch j+3 whole tiles on the sync stream
                if j + 3 < NJ:
                    for slot in (0, 1):
                        dma_w13(slot, 0, j + 3)
                        dma_w13(slot, 1, j + 3)
                if j + 1 < NJ:
                    dma_w2a(j + 1)

                for slot, _, cw in SLOTS:
                    gu_step(slot, j)
                # fold down-proj chunks 0..NC1-1 for act[j-1] into this step
                fold_slot(0, j - 1)
                if j - 1 >= 4:
                    fold_slot(1, j - 1)
                if j == 4:
                    # deferred slot-B j0/j1 passes + their catch-up folds
                    gu_step(1, 0)
                    gu_step(1, 1)
                    for jb in range(4):
                        fold_slot(1, jb)

            # w2b chunk tiles: emitted after the whole w13 stream; the sync
            # queue reaches them as the w13 lead drains, prefetching the
            # first 16 during late GU and streaming the rest just-in-time
            # through DOWN. Order matches the interleaved DOWN pass order.
            for h in range(NC1, NH):
                for slot in (1, 0):
                    dma_w2bh(slot, h)

            def writeback(ps, h, slot, c0, cw, eng=None):
                o_sb = outp.tile([P, cw], BF16, tag="o", name=f"o_{slot}_{h}")
                nc.vector.tensor_copy(o_sb[:], ps[:])
                # gpsimd (software DGE) keeps output writebacks off the HW
                # input queues; the final chunks go on sync (stream empty by
                # then; faster drain).
                (eng or nc.gpsimd).dma_start(y_d[h][:, c0:c0 + cw], o_sb[:])

            # finish the interleaved accumulators (act[NJ-1]) and drain them
            for slot, c0, cw in SLOTS:
                for c in range(NC1):
                    nc.tensor.matmul(
                        acc[(slot, c)][:],
                        w2a_tiles[NJ - 1][:, slot, c * P:(c + 1) * P],
                        act_tiles[(slot, NJ - 1)][:],
                        start=False,
                        stop=True,
                    )
            for slot, c0, cw in SLOTS:
                for c in range(NC1):
                    writeback(acc[(slot, c)], c, slot, c0, cw)

            # ---- DOWN: remaining chunks, interleaved B,A per h so the
            # per-core w2bh demand is steady (~273 GB/s) instead of peaking
            # at 299 through an all-B phase — with all 8 cores in lockstep,
            # that peak starved the slowest core's JIT stream and tripped
            # its clock-gate. 6-wide PSUM cycling per slot as before.
            hi_per_slot = {0: 0, 1: 0}
            for h in range(NC1, NH):
                for slot, c0, cw in ((1, CA, CB), (0, 0, CA)):
                    hi = hi_per_slot[slot]
                    hi_per_slot[slot] += 1
                    tag_cycle = ["g", "g", "u", "u", f"acc{slot}0", f"acc{slot}1"]
                    ps = (psgu if hi % 6 < 4 else psacc).tile(
                        [P, cw], F32, tag=tag_cycle[hi % 6], name=f"yd_{slot}_{h}"
                    )
                    for j in range(NJ):
                        nc.tensor.matmul(
                            ps[:],
                            w2bh_tiles[(slot, h)][:, j * P:(j + 1) * P],
                            act_tiles[(slot, j)][:],
                            start=(j == 0),
                            stop=(j == NJ - 1),
                        )
                    writeback(ps, h, slot, c0, cw,
                              eng=nc.sync if h >= NH - 3 else None)
    nc.compile()
    return nc


def _route(router_logits: np.ndarray, top_k: int):
    """Match jax.nn.softmax + jax.lax.top_k + renormalize (ties -> lower idx)."""
    p = router_logits.astype(np.float64)
    p = np.exp(p - p.max(axis=-1, keepdims=True))
    p /= p.sum(axis=-1, keepdims=True)
    order = np.argsort(-p, axis=-1, kind="stable")
    idx = order[:, :top_k]
    w = np.take_along_axis(p, idx, axis=-1)
    w /= w.sum(axis=-1, keepdims=True)
    return idx, w


def _pad4(n: int) -> int:
    return max(16, -(-n // 4) * 4)


def kernel(hidden_states, router_logits, W13, W2, top_k):
    global last_results
    top_k = int(top_k)
    hs = np.asarray(hidden_states, dtype=np.float32)
    T = hs.shape[0]
    idx, w = _route(np.asarray(router_logits, dtype=np.float32), top_k)

    tok_ids, tok_w = [], []
    for e in range(E):
        sel = idx == e  # [T, k]; at most one True per row
        rows = np.nonzero(sel.any(axis=-1))[0]
        tok_ids.append(rows)
        tok_w.append(w[sel].astype(np.float32))  # row-major -> token order

    # sort experts by load desc; pair big (slot A) with small (slot B)
    loads = np.array([len(r) for r in tok_ids])
    order = np.argsort(-loads, kind="stable")
    pairs = [(int(order[p]), int(order[7 - p])) for p in range(4)]
    CA = _pad4(max(loads[a] for a, _ in pairs))
    CB = _pad4(max(loads[b] for _, b in pairs))
    assert CA <= 512 and CB <= 512, "token capacity exceeds one PSUM bank"
    CT = CA + CB

    W13 = np.asarray(W13, dtype=np.float32)
    W2 = np.asarray(W2, dtype=np.float32)
    hsb = hs.astype(ml_dtypes.bfloat16)

    def w13_shard(e, hf):
        # [gate-half; up-half] rows -> [NJ, 2, P, KO, P] tiled, partition=h-col
        wg = W13[e][hf * IH:(hf + 1) * IH]
        wu = W13[e][I + hf * IH:I + (hf + 1) * IH]
        both = np.concatenate([wg, wu], axis=0).astype(ml_dtypes.bfloat16)
        # rows [2*IH] -> (kind, j, P); transpose to [j, kind, P(h), KO, P(row)]
        t = both.reshape(2, NJ, P, KO, P).transpose(1, 0, 4, 3, 2)
        return np.ascontiguousarray(t)

    def w2_shard(e, hf):
        # contraction rows i within the half -> [NJ, P, H]
        w2h = W2[e][:, hf * IH:(hf + 1) * IH].astype(ml_dtypes.bfloat16)
        return np.ascontiguousarray(w2h.reshape(H, NJ, P).transpose(1, 2, 0))

    in_maps = []
    for core in range(N_CORES):
        p, hf = core // 2, core % 2
        ea, eb = pairs[p]

        def xt_arr(e, cap):
            xt = np.zeros((P, KO, cap), dtype=ml_dtypes.bfloat16)
            rows = tok_ids[e]
            n_e = len(rows)
            if n_e:
                xg = hsb[rows]  # [n_e, H]
                xt[:, :, :n_e] = xg.T.reshape(KO, P, n_e).transpose(1, 0, 2)
            return xt

        w2 = np.stack([w2_shard(ea, hf), w2_shard(eb, hf)])  # [2, NJ, P, H]
        # w2bh: [2, NHD, P, NJ*P] — per-(slot, h) tiles with all NJ slices
        w2bh = np.ascontiguousarray(
            w2[:, :, :, NC1 * P:].reshape(2, NJ, P, NHD, P)
            .transpose(0, 3, 2, 1, 4).reshape(2, NHD, P, NJ * P)
        )
        in_maps.append({
            "xta": xt_arr(ea, CA),
            "xtb": xt_arr(eb, CB),
            "w13a": w13_shard(ea, hf),
            "w13b": w13_shard(eb, hf),
            # [NJ, P, 2, NC1*P]: both slots packed per j
            "w2a": np.ascontiguousarray(w2[:, :, :, :NC1 * P].transpose(1, 2, 0, 3)),
            "w2bh": w2bh,
        })

    nc = _build_nc(CA, CB)
    res = run_bass_kernel_spmd(
        nc,
        in_maps,
        list(range(N_CORES)),
        trace=bool(os.environ.get("MOE_TRACE")),
        tmpdir=os.environ.get("MOE_TRACE_DIR") or None,
    )
    last_results = res

    out = np.zeros((T, H), dtype=np.float32)
    for p in range(4):
        ea, eb = pairs[p]
        y0 = res.results[2 * p]["y"].reshape(H, CT).astype(np.float32)
        y1 = res.results[2 * p + 1]["y"].reshape(H, CT).astype(np.float32)
        ysum = y0 + y1
        for (e, c0) in ((ea, 0), (eb, CA)):
            rows = tok_ids[e]
            n_e = len(rows)
            if n_e:
                out[rows] += ysum[:, c0:c0 + n_e].T * tok_w[e][:, None]
    return out


# revision 49
# speedup vs baseline: 1.1730x; 1.1730x over previous
"""Expert-parallel MoE (Mixtral-style top-2 of 8 experts, SwiGLU) on 8 TRN2 cores.

Strategy: expert PAIRING with a half-intermediate split. Experts are sorted by
token load and paired big-with-small; the two cores of pair p each process ALL
tokens of both experts, but only half of the intermediate dim I (so weight
bytes per core stay at 1/8 of the total). The host sums the two half-I
down-proj partials per expert. Slot capacities are uniform across cores (SPMD
single program): CA = max big-expert load, CB = max small-expert load — this
equals the packing lower bound l1+l5.

Routing (softmax/top-k/renorm, 1024x8) runs on host during input sharding;
the renormalized routing weight is folded in at host combine time.

Device kernel per core (all matmuls bf16, fp32 PSUM accumulation):
  GU:   for j, for slot in (A,B): g/u[j] = W13T[slot,j] . xt  (16 ko steps)
        act = silu(g)*u  (bf16); 2 PSUM accumulators per slot fold the first
        2 down-proj output chunks in as act[j-1] becomes available
  DOWN: remaining 14 output chunks per slot, B/A interleaved per h-chunk,
        6-wide PSUM passes (reusing the g/u + accumulator banks); output
        written bf16. The interleave keeps every core's w2b stream demand
        steady (~273 GB/s) — an all-B phase peaks at 299 GB/s on all 8
        cores in lockstep, which starved the slowest core's JIT stream,
        stalled its PE >3.4us and tripped the HAM clock-gate back to
        1.2 GHz (exec time is the MAX over cores, so one straggler core
        is the whole metric).

DMA discipline (the per-core limits that shape this kernel):
  - All DMA queues share the same 16 DMA engines (~400 GB/s aggregate);
    concurrently active queues split that roughly evenly, so a sustained
    second stream starves the critical one. Only sync and scalar have HW
    DGE queues; gpsimd's software DGE does ~50 GB/s.
  - Both GU and DOWN consume weights at ~256 GB/s (each weight byte feeds
    one matmul of ~280 columns). Total input is 51 MB against a ~182 us
    matmul stream — the kernel is near BOTH the PE and the single-queue
    DMA roofline, so the sync queue carries ALL weights in exact
    consumption order and must never idle:
      w13 j-tiles (4/j) -> w2b as per-(slot,h) chunk tiles streamed
      just-in-time through DOWN (pool backpressure paces them), slot B
      chunks first to match the DOWN order.
  - The w13 pool holds 16 tiles (4 j-groups of lead) so transient DMA dips
    never stall the PE; w2b-by-h tiles need only 8 bufs (32 KB/partition)
    instead of a fully-resident 112 KB/partition prefetch.
  - The start is slot-A-first so the PE never waits for the full xt: xta
    blocks + A weights (~2.8 MB) feed j0A/j1A while xtb + B weights stream
    behind them; xt rides scalar (active only the first ~10 us), and the
    j0/j1 weight tiles arrive as 8-ko halves so each piece is usable as
    soon as its prefix of the sync stream lands. w2a (per-j 128 KB tiles
    for the folded chunks) and output writebacks ride gpsimd's software
    DGE (~13 GB/s trickle).
  - The first ~10 us are DMA-ramp-bound (all 8 cores slam HBM at once), so
    dep-free warm-up matmuls on a memset tile run upfront (N_WARM) and as
    filler batches inside the j0A gate phase (J0_FILL): in-order PE
    execution places them exactly in the DMA-wait windows, keeping the PE
    busy so the HAM clock-gate holds 2.4 GHz from ~11 us for the whole
    kernel (otherwise ramp gaps re-throttle it to 1.2 GHz).
"""

import os

import ml_dtypes
import numpy as np

import concourse.bass as bass
from concourse import bacc
import concourse.mybir as mybir
import concourse.tile as tile
from concourse.bass_utils import run_bass_kernel_spmd

P = 128
H = 2048          # hidden dim
I = 4096          # intermediate dim
IH = I // 2       # per-core intermediate half
E = 8             # experts
N_CORES = 8
BF16 = mybir.dt.bfloat16
F32 = mybir.dt.float32

KO = H // P       # 16 contraction steps over hidden dim
NJ = IH // P      # 16 j-tiles per slot (half intermediate)
NH = H // P       # 16 output row chunks
NC1 = 2           # down-proj chunks folded into the GU phase per slot
NHD = NH - NC1    # down-proj chunks done in the DOWN phase
XB = 4            # xt ko-blocks (KO/XB per block)
KB = KO // XB     # ko per block (4)
N_WARM = 16       # warm-up dummy matmuls (HAM un-throttle during DMA prime)

# set by kernel() for test harness introspection
last_results = None


def _build_nc(CA: int, CB: int) -> bass.Bass:
    act_fn = mybir.ActivationFunctionType
    CT = CA + CB

    nc = bacc.Bacc()
    xta_d = nc.declare_dram_parameter("xta", [P, KO, CA], BF16, isOutput=False)
    xtb_d = nc.declare_dram_parameter("xtb", [P, KO, CB], BF16, isOutput=False)
    # per slot: [j, kind(0=gate,1=up), P, KO, P]
    w13a_d = nc.declare_dram_parameter("w13a", [NJ, 2, P, KO, P], BF16, isOutput=False)
    w13b_d = nc.declare_dram_parameter("w13b", [NJ, 2, P, KO, P], BF16, isOutput=False)
    # w2a: per-j tiles for the NC1 folded chunks; w2bh: per-(slot, h) tiles
    # holding all NJ contraction slices for one output chunk
    w2a_d = nc.declare_dram_parameter("w2a", [NJ, P, 2, NC1 * P], BF16, isOutput=False)
    w2bh_d = nc.declare_dram_parameter("w2bh", [2, NHD, P, NJ * P], BF16, isOutput=False)
    y_d = nc.declare_dram_parameter("y", [NH, P, CT], BF16, isOutput=True)

    SLOTS = [(0, 0, CA), (1, CA, CB)]  # (slot, col offset, width)
    w13_dram = (w13a_d, w13b_d)

    with tile.TileContext(nc) as tc:
        with (
            tc.tile_pool(name="xtp", bufs=1) as xtp,
            tc.tile_pool(name="w13f", bufs=8) as w13fp,
            tc.tile_pool(name="w13p", bufs=12) as w13p,
            tc.tile_pool(name="w2ap", bufs=4) as w2ap,
            tc.tile_pool(name="w2bp", bufs=16) as w2bp,
            tc.tile_pool(name="actp", bufs=1) as actp,
            tc.tile_pool(name="silup", bufs=2) as silup,
            tc.tile_pool(name="outp", bufs=4) as outp,
            tc.tile_pool(name="dummyp", bufs=1) as dummyp,
            tc.tile_pool(name="psgu", bufs=2, space="PSUM") as psgu,
            tc.tile_pool(name="psacc", bufs=1, space="PSUM") as psacc,
        ):
            # ---- tiles and DMA helpers ------------------------------------
            xta_tiles, xtb_tiles = [], []

            def dma_xta(b):
                sb = xtp.tile([P, KB, CA], BF16, tag=f"xta_{b}", name=f"xta_{b}")
                nc.scalar.dma_start(sb[:], xta_d[:, b * KB:(b + 1) * KB, :])
                assert len(xta_tiles) == b
                xta_tiles.append(sb)

            def dma_xtb(hf):
                kb = KO // 2
                sb = xtp.tile([P, kb, CB], BF16, tag=f"xtb_{hf}", name=f"xtb_{hf}")
                nc.scalar.dma_start(sb[:], xtb_d[:, hf * kb:(hf + 1) * kb, :])
                assert len(xtb_tiles) == hf
                xtb_tiles.append(sb)

            def xt_slice(ko, slot):
                if slot == 0:
                    return xta_tiles[ko // KB][:, ko % KB, :]
                kb = KO // 2
                return xtb_tiles[ko // kb][:, ko % kb, :]

            # j0/j1 tiles are halved (8-ko pieces) so each piece is usable
            # as soon as its prefix of the sync stream lands
            HALVED = {(s, k, j) for s in (0, 1) for k in (0, 1) for j in (0, 1)}
            w13h = {}

            def dma_w13h(slot, kind, j, hf):
                kb = KO // 2
                sb = w13fp.tile([P, kb, P], BF16, tag="w13h",
                                name=f"w13h_{slot}_{kind}_{j}_{hf}")
                nc.sync.dma_start(
                    sb[:], w13_dram[slot][j, kind][:, hf * kb:(hf + 1) * kb, :])
                w13h[(slot, kind, j, hf)] = sb

            w13_tiles = {}

            def dma_w13(slot, kind, j):
                sb = w13p.tile([P, KO, P], BF16, tag="w13",
                               name=f"w13_{slot}_{kind}_{j}")
                nc.sync.dma_start(sb[:], w13_dram[slot][j, kind])
                w13_tiles[(slot, kind, j)] = sb

            w2a_tiles = {}

            def dma_w2a(j):
                sb = w2ap.tile([P, 2, NC1 * P], BF16, tag="w2a", name=f"w2a_{j}")
                nc.gpsimd.dma_start(sb[:], w2a_d[j])
                w2a_tiles[j] = sb

            w2bh_tiles = {}

            def dma_w2bh(slot, h):
                # one tile per DOWN pass; bufs=8 backpressure paces the sync
                # queue just-in-time through the DOWN phase
                sb = w2bp.tile([P, NJ * P], BF16, tag="w2bh", name=f"w2bh_{slot}_{h}")
                nc.sync.dma_start(sb[:], w2bh_d[slot, h - NC1])
                w2bh_tiles[(slot, h)] = sb

            # ---- warm-up tile (memset, no DMA needed) ---------------------
            dummy = dummyp.tile([P, CA], BF16, tag="dummy", name="dummy")
            nc.vector.memset(dummy[:], 0.0)

            # ---- priming -------------------------------------------------
            # scalar: xta blocks then xtb halves (idle afterwards); sync:
            # A-slot j0/j1 weights first, then B-slot, then the j2+ stream
            # in consumption order; gpsimd: w2a per-j tiles.
            dma_xta(0)
            dma_w13h(0, 0, 0, 0)     # Ag j0 first half (256 KB)
            dma_xta(1)
            dma_w13h(0, 0, 0, 1)
            dma_xta(2)
            dma_xta(3)
            dma_w13h(0, 1, 0, 0)     # Au j0 halves
            dma_w13h(0, 1, 0, 1)
            dma_xtb(0)
            dma_w13h(0, 0, 1, 0)     # j1 A halves
            dma_w13h(0, 0, 1, 1)
            dma_w13h(0, 1, 1, 0)
            dma_w13h(0, 1, 1, 1)
            dma_xtb(1)
            for j in (2, 3, 4):      # deep prime; loop prefetches from j5
                dma_w13(0, 0, j)
                dma_w13(0, 1, j)
                dma_w13(1, 0, j)
                dma_w13(1, 1, j)
            # B j0/j1 halves trail the prime: their passes are DEFERRED to
            # mid-GU (after j4), so these land in the steady stream instead
            # of widening the startup DMA crunch
            for (k, j) in ((0, 0), (1, 0), (0, 1), (1, 1)):
                dma_w13h(1, k, j, 0)
                dma_w13h(1, k, j, 1)
            dma_w2a(0)               # gpsimd, per-j 128 KB tiles
            dma_w2a(1)
            dma_w2a(2)

            # persistent PSUM accumulators for the first NC1 output chunks
            acc = {}
            for slot, _, cw in SLOTS:
                for c in range(NC1):
                    acc[(slot, c)] = psacc.tile(
                        [P, cw], F32, tag=f"acc{slot}{c}", name=f"acc_{slot}_{c}"
                    )

            # ---- warm-up dummies: keep PE busy from ~7us so HAM is at
            # 2.4 GHz when the real stream starts; they write acc00 which the
            # j0 fold later resets with start=True.
            for w in range(N_WARM):
                nc.tensor.matmul(
                    acc[(0, 0)][:], dummy[:, 0:P], dummy[:],
                    start=True, stop=True,
                )

            act_tiles = {}

            def silu(slot, j, g_ps, u_ps, cw):
                s_sb = silup.tile([P, cw], F32, tag="s", name=f"s_{slot}_{j}")
                nc.scalar.activation(s_sb[:], g_ps[:], act_fn.Sigmoid)
                su_sb = silup.tile([P, cw], F32, tag="su", name=f"su_{slot}_{j}")
                nc.vector.tensor_mul(su_sb[:], s_sb[:], u_ps[:])
                a_sb = actp.tile([P, cw], BF16, tag=f"act_{slot}_{j}",
                                 name=f"act_{slot}_{j}")
                nc.vector.tensor_mul(a_sb[:], su_sb[:], g_ps[:])
                act_tiles[(slot, j)] = a_sb

            def warm_batch(n):
                # dep-free filler matmuls: in-order PE execution runs them
                # exactly during early DMA waits, keeping HAM at 2.4 GHz
                for _ in range(n):
                    nc.tensor.matmul(
                        acc[(0, 0)][:], dummy[:, 0:P], dummy[:],
                        start=True, stop=True,
                    )

            # filler sizes inside j0A after (kind, ko) — the xt-block and
            # weight-half wait points of the DMA ramp
            J0_FILL = {(0, 3): 14, (0, 7): 8, (0, 11): 5, (0, 15): 5,
                       (1, 3): 6, (1, 11): 4}

            def gu_step(slot, j):
                _, c0, cw = SLOTS[slot]
                g_ps = psgu.tile([P, cw], F32, tag="g", name=f"g_{slot}_{j}")
                u_ps = psgu.tile([P, cw], F32, tag="u", name=f"u_{slot}_{j}")
                kb = KO // 2
                for kind, ps in ((0, g_ps), (1, u_ps)):
                    for ko in range(KO):
                        if (slot, kind, j) in HALVED:
                            w_sl = w13h[(slot, kind, j, ko // kb)][:, ko % kb, :]
                        else:
                            w_sl = w13_tiles[(slot, kind, j)][:, ko, :]
                        nc.tensor.matmul(
                            ps[:], w_sl, xt_slice(ko, slot),
                            start=(ko == 0), stop=(ko == KO - 1),
                        )
                        if slot == 0 and j == 0:
                            warm_batch(J0_FILL.get((kind, ko), 0))
                silu(slot, j, g_ps, u_ps, cw)

            def fold_slot(slot, j, stop=False):
                for c in range(NC1):
                    nc.tensor.matmul(
                        acc[(slot, c)][:],
                        w2a_tiles[j][:, slot, c * P:(c + 1) * P],
                        act_tiles[(slot, j)][:],
                        start=(j == 0),
                        stop=stop,
                    )

            # ---- GU start: slot A only (just xta + A weight halves on the
            # critical path); slot B's j0/j1 are deferred to after j4, by
            # which time the DMA ramp is over and the stream is PE-bound
            gu_step(0, 0)
            gu_step(0, 1)
            fold_slot(0, 0)

            # ---- GU j = 2..NJ-1 ------------------------------------------
            for j in range(2, NJ):
                # prefetch j+3 whole tiles on the sync stream
                if j + 3 < NJ:
                    for slot in (0, 1):
                        dma_w13(slot, 0, j + 3)
                        dma_w13(slot, 1, j + 3)
                if j + 1 < NJ:
                    dma_w2a(j + 1)

                for slot, _, cw in SLOTS:
                    gu_step(slot, j)
                # fold down-proj chunks 0..NC1-1 for act[j-1] into this step
                fold_slot(0, j - 1)
                if j - 1 >= 4:
                    fold_slot(1, j - 1)
                if j == 4:
                    # deferred slot-B j0/j1 passes + their catch-up folds
                    gu_step(1, 0)
                    gu_step(1, 1)
                    for jb in range(4):
                        fold_slot(1, jb)

            # w2b chunk tiles: emitted after the whole w13 stream; the sync
            # queue reaches them as the w13 lead drains, prefetching the
            # first 16 during late GU and streaming the rest just-in-time
            # through DOWN. Order matches the interleaved DOWN pass order.
            for h in range(NC1, NH):
                for slot in (1, 0):
                    dma_w2bh(slot, h)

            def writeback(ps, h, slot, c0, cw, eng=None):
                o_sb = outp.tile([P, cw], BF16, tag="o", name=f"o_{slot}_{h}")
                nc.vector.tensor_copy(o_sb[:], ps[:])
                # gpsimd (software DGE) keeps output writebacks off the HW
                # input queues; the final chunks go on sync (stream empty by
                # then; faster drain).
                (eng or nc.gpsimd).dma_start(y_d[h][:, c0:c0 + cw], o_sb[:])

            # finish the interleaved accumulators (act[NJ-1]) and drain them
            for slot, c0, cw in SLOTS:
                for c in range(NC1):
                    nc.tensor.matmul(
                        acc[(slot, c)][:],
                        w2a_tiles[NJ - 1][:, slot, c * P:(c + 1) * P],
                        act_tiles[(slot, NJ - 1)][:],
                        start=False,
                        stop=True,
                    )
            for slot, c0, cw in SLOTS:
                for c in range(NC1):
                    writeback(acc[(slot, c)], c, slot, c0, cw)

            # ---- DOWN: remaining chunks, interleaved B,A per h so the
            # per-core w2bh demand is steady (~273 GB/s) instead of peaking
            # at 299 through an all-B phase — with all 8 cores in lockstep,
            # that peak starved the slowest core's JIT stream and tripped
            # its clock-gate. 6-wide PSUM cycling per slot as before.
            hi_per_slot = {0: 0, 1: 0}
            for h in range(NC1, NH):
                for slot, c0, cw in ((1, CA, CB), (0, 0, CA)):
                    hi = hi_per_slot[slot]
                    hi_per_slot[slot] += 1
                    tag_cycle = ["g", "g", "u", "u", f"acc{slot}0", f"acc{slot}1"]
                    ps = (psgu if hi % 6 < 4 else psacc).tile(
                        [P, cw], F32, tag=tag_cycle[hi % 6], name=f"yd_{slot}_{h}"
                    )
                    last = slot == 0 and h == NH - 1
                    # the very last pass runs in two column halves so the
                    # first half's writeback overlaps the second half's
                    # matmuls (shortens the serial drain after the last MM)
                    col_splits = ((0, cw // 2), (cw // 2, cw - cw // 2)) \
                        if last else ((0, cw),)
                    for cs, cn in col_splits:
                        for j in range(NJ):
                            nc.tensor.matmul(
                                ps[:, cs:cs + cn],
                                w2bh_tiles[(slot, h)][:, j * P:(j + 1) * P],
                                act_tiles[(slot, j)][:, cs:cs + cn],
                                start=(j == 0),
                                stop=(j == NJ - 1),
                            )
                        if last:
                            o_sb = outp.tile([P, cn], BF16, tag="o",
                                             name=f"o_{slot}_{h}_{cs}")
                            nc.vector.tensor_copy(o_sb[:], ps[:, cs:cs + cn])
                            nc.sync.dma_start(
                                y_d[h][:, c0 + cs:c0 + cs + cn], o_sb[:])
                    if not last:
                        writeback(ps, h, slot, c0, cw,
                                  eng=nc.sync if h >= NH - 3 else None)
    nc.compile()
    return nc


def _route(router_logits: np.ndarray, top_k: int):
    """Match jax.nn.softmax + jax.lax.top_k + renormalize (ties -> lower idx)."""
    p = router_logits.astype(np.float64)
    p = np.exp(p - p.max(axis=-1, keepdims=True))
    p /= p.sum(axis=-1, keepdims=True)
    order = np.argsort(-p, axis=-1, kind="stable")
    idx = order[:, :top_k]
    w = np.take_along_axis(p, idx, axis=-1)
    w /= w.sum(axis=-1, keepdims=True)
    return idx, w


def _pad4(n: int) -> int:
    return max(16, -(-n // 4) * 4)


def kernel(hidden_states, router_logits, W13, W2, top_k):
    global last_results
    top_k = int(top_k)
    hs = np.asarray(hidden_states, dtype=np.float32)
    T = hs.shape[0]
    idx, w = _route(np.asarray(router_logits, dtype=np.float32), top_k)

    tok_ids, tok_w = [], []
    for e in range(E):
        sel = idx == e  # [T, k]; at most one True per row
        rows = np.nonzero(sel.any(axis=-1))[0]
        tok_ids.append(rows)
        tok_w.append(w[sel].astype(np.float32))  # row-major -> token order

    # sort experts by load desc; pair big (slot A) with small (slot B)
    loads = np.array([len(r) for r in tok_ids])
    order = np.argsort(-loads, kind="stable")
    pairs = [(int(order[p]), int(order[7 - p])) for p in range(4)]
    CA = _pad4(max(loads[a] for a, _ in pairs))
    CB = _pad4(max(loads[b] for _, b in pairs))
    assert CA <= 512 and CB <= 512, "token capacity exceeds one PSUM bank"
    CT = CA + CB

    W13 = np.asarray(W13, dtype=np.float32)
    W2 = np.asarray(W2, dtype=np.float32)
    hsb = hs.astype(ml_dtypes.bfloat16)

    def w13_shard(e, hf):
        # [gate-half; up-half] rows -> [NJ, 2, P, KO, P] tiled, partition=h-col
        wg = W13[e][hf * IH:(hf + 1) * IH]
        wu = W13[e][I + hf * IH:I + (hf + 1) * IH]
        both = np.concatenate([wg, wu], axis=0).astype(ml_dtypes.bfloat16)
        # rows [2*IH] -> (kind, j, P); transpose to [j, kind, P(h), KO, P(row)]
        t = both.reshape(2, NJ, P, KO, P).transpose(1, 0, 4, 3, 2)
        return np.ascontiguousarray(t)

    def w2_shard(e, hf):
        # contraction rows i within the half -> [NJ, P, H]
        w2h = W2[e][:, hf * IH:(hf + 1) * IH].astype(ml_dtypes.bfloat16)
        return np.ascontiguousarray(w2h.reshape(H, NJ, P).transpose(1, 2, 0))

    in_maps = []
    for core in range(N_CORES):
        p, hf = core // 2, core % 2
        ea, eb = pairs[p]

        def xt_arr(e, cap):
            xt = np.zeros((P, KO, cap), dtype=ml_dtypes.bfloat16)
            rows = tok_ids[e]
            n_e = len(rows)
            if n_e:
                xg = hsb[rows]  # [n_e, H]
                xt[:, :, :n_e] = xg.T.reshape(KO, P, n_e).transpose(1, 0, 2)
            return xt

        w2 = np.stack([w2_shard(ea, hf), w2_shard(eb, hf)])  # [2, NJ, P, H]
        # w2bh: [2, NHD, P, NJ*P] — per-(slot, h) tiles with all NJ slices
        w2bh = np.ascontiguousarray(
            w2[:, :, :, NC1 * P:].reshape(2, NJ, P, NHD, P)
            .transpose(0, 3, 2, 1, 4).reshape(2, NHD, P, NJ * P)
        )
        in_maps.append({
            "xta": xt_arr(ea, CA),
            "xtb": xt_arr(eb, CB),
            "w13a": w13_shard(ea, hf),
            "w13b": w13_shard(eb, hf),
            # [NJ, P, 2, NC1*P]: both slots packed per j
            "w2a": np.ascontiguousarray(w2[:, :, :, :NC1 * P].transpose(1, 2, 0, 3)),
            "w2bh": w2bh,
        })

    nc = _build_nc(CA, CB)
    res = run_bass_kernel_spmd(
        nc,
        in_maps,
        list(range(N_CORES)),
        trace=bool(os.environ.get("MOE_TRACE")),
        tmpdir=os.environ.get("MOE_TRACE_DIR") or None,
    )
    last_results = res

    out = np.zeros((T, H), dtype=np.float32)
    for p in range(4):
        ea, eb = pairs[p]
        y0 = res.results[2 * p]["y"].reshape(H, CT).astype(np.float32)
        y1 = res.results[2 * p + 1]["y"].reshape(H, CT).astype(np.float32)
        ysum = y0 + y1
        for (e, c0) in ((ea, 0), (eb, CA)):
            rows = tok_ids[e]
            n_e = len(rows)
            if n_e:
                out[rows] += ysum[:, c0:c0 + n_e].T * tok_w[e][:, None]
    return out
